# revision 18
# baseline (speedup 1.0000x reference)
"""Trainium2 Bass kernel for nn_MultiHeadedAttention (B=2, H=16, S=2048, d=64).

Sharding: data-parallel over batch x tensor-parallel over heads.
8 cores = 2 batch groups x 4 head-groups (4 heads each).

v5 schedule (traced-informed rewrite of the 175us baseline):
  - Startup is HBM-bound (~5.6MB of inputs): weights host-pre-arranged into
    contiguous SBUF layouts, loads split across both HWDGE rings ordered by
    first use; wo bf16 + deferred.  Prefix: 8 projection tiles (K/Q hp0 +
    V st0-3) accumulate in 8 parallel PSUM banks, kc loop outermost, so each
    xT chunk is consumed on arrival.  PE warm-up zero-MMs (HAM -> 8/8) reuse
    the prefix tiles; a dummy exp preloads the ACT table.
  - The attention inner loop is ACT-bound (~1.3-1.5us per 512-chunk of exp
    vs ~0.6us of PE work), so projection/O-proj matmuls are interleaved into
    the emission at a measured per-chunk byte budget: each chunk banks
    (act_time - pe_time) and pops filler items when it can afford them.
    Filler order is chosen so every dependency (kT/qT/v tiles, hp1 weights)
    lands just before the attention phase that consumes it.
  - hs->hsT transposes ride the DMA xbar on the otherwise-idle sync ring
    (no PE, no PSUM, no DVE); output DMAs go on the scalar ring to keep the
    sync ring xbar-mode-pure.
  - V filler tiles are region-shared two-per-PSUM-bank (zero-prefill +
    start=False accumulation, same trick as the attention slots) to halve
    bank handoffs through the single filler bank.
  - Leftover O-proj drains into a 4-wide tail pool after the attention pools
    close, with copies alternating VectorE/ScalarE (ACT is free by then).
  - Output shipped bf16; in-loop PSUM->SBUF copies pinned to VectorE.
Attention math is identical to the baseline (see kernel_baseline.py):
fp32->bf16 projections, 2-head row-group-packed score matmuls, one exp per
(kt, 512-chunk) covering both heads (scale=1/8, no max subtraction),
tri-mask on diagonal blocks, PV accumulation with a ones column for the
softmax denominator, eager reciprocal+scale normalization at kt==jq.
Host adds the exact (b_V @ W_O + b_O) row.
"""

import math
from collections import deque
from contextlib import ExitStack

import numpy as np
import ml_dtypes

import concourse.bass as bass
import concourse.mybir as mybir
import concourse.tile as tile
from concourse import bacc, bass_utils

F32 = mybir.dt.float32
BF16 = mybir.dt.bfloat16
EXP = mybir.ActivationFunctionType.Exp

B, S, D = 2, 2048, 1024
NH, HD = 16, 64
NCORES = 8
GROUPS = NCORES // B          # 4 head-groups per batch
HPC = NH // GROUPS            # 4 heads per core
M = HPC * HD                  # 256 local head-dims per core
P = 128
KC = D // P                   # 8 contraction chunks
NT = S // P                   # 16 q/s tiles
SCALE = 1.0 / math.sqrt(HD)   # 0.125
N_WARMUP = 8

QK_COST = 1752                # 8 N=512 matmuls
VPAIR_COST = 2021             # prefill + 16 N=256 matmuls
OP_COST = 438                 # 2 N=512 matmuls
BUDGET_CAP = 6000.0


def build_kernel():
    nc = bacc.Bacc("TRN2", target_bir_lowering=False)

    xT_d = nc.dram_tensor("xT", [D, S], BF16, kind="ExternalInput")
    wq_d = nc.dram_tensor("wq", [P, KC, M], BF16, kind="ExternalInput")
    wk_d = nc.dram_tensor("wk", [P, KC, M], BF16, kind="ExternalInput")
    wv_d = nc.dram_tensor("wv", [P, KC, M], BF16, kind="ExternalInput")
    wo_d = nc.dram_tensor("wo", [P, 2, D], BF16, kind="ExternalInput")
    bq_d = nc.dram_tensor("bq", [P, 2], F32, kind="ExternalInput")
    bk_d = nc.dram_tensor("bk", [P, 2], F32, kind="ExternalInput")
    tri_d = nc.dram_tensor("tri", [P, P], BF16, kind="ExternalInput")
    out_d = nc.dram_tensor("out", [S, D], BF16, kind="ExternalOutput")

    with tile.TileContext(nc) as tc, ExitStack() as ctx:
        big = ctx.enter_context(tc.tile_pool(name="big", bufs=1))
        exp_pool = ctx.enter_context(tc.tile_pool(name="expp", bufs=8))
        outcp = ctx.enter_context(tc.tile_pool(name="outcp", bufs=4))
        recip_pool = ctx.enter_context(tc.tile_pool(name="recipp", bufs=2))

        # ---- persistent SBUF tiles ----
        xT_sb = big.tile([P, KC, S], BF16)
        wq_sb = big.tile([P, KC, M], BF16)
        wk_sb = big.tile([P, KC, M], BF16)
        wv_sb = big.tile([P, KC, M], BF16)
        wo_sb = big.tile([P, 2, D], BF16)
        bq_sb = big.tile([P, 2], F32)
        bk_sb = big.tile([P, 2], F32)
        qT_sb = big.tile([P, 2, S], BF16)
        kT_sb = big.tile([P, 2, S], BF16)
        v_sb = big.tile([P, NT, HPC, HD + 1], BF16)
        hs_sb = big.tile([P, NT, M], BF16)
        hsT_sb = big.tile([P, 2, NT, P], BF16)
        tri_sb = big.tile([P, P], BF16)
        zz_sb = big.tile([1, 512], BF16)
        dummy_sb = big.tile([1, 2], BF16)

        nc.vector.memset(zz_sb[:], 0.0)
        nc.vector.memset(v_sb[:, :, :, HD : HD + 1], 1.0)
        # preload the ACT exp table (~2.7us) during the DMA prefix
        nc.scalar.activation(dummy_sb[:], zz_sb[0:1, 0:2], EXP, scale=SCALE)

        # ---- input DMAs: contiguous slabs on both rings, by first use ----
        nc.sync.dma_start(wq_sb[:], wq_d.ap())
        nc.scalar.dma_start(wk_sb[:], wk_d.ap())
        nc.scalar.dma_start(bk_sb[:], bk_d.ap())
        nc.scalar.dma_start(bq_sb[:], bq_d.ap())
        nc.scalar.dma_start(wv_sb[:], wv_d.ap())
        for kc in range(KC):
            eng = nc.sync if kc % 2 == 0 else nc.scalar
            eng.dma_start(xT_sb[:, kc, :], xT_d.ap()[P * kc : P * (kc + 1), :])
        nc.sync.dma_start(tri_sb[:], tri_d.ap())
        nc.scalar.dma_start(wo_sb[:], wo_d.ap())

        # ---- projection building blocks ----
        def qk_mm(ps, which, hp, nq, kc):
            w_sb = wq_sb if which == "q" else wk_sb
            nc.tensor.matmul(
                ps[:],
                lhsT=w_sb[:, kc, P * hp : P * (hp + 1)],
                rhs=xT_sb[:, kc, 512 * nq : 512 * (nq + 1)],
                start=(kc == 0),
                stop=(kc == KC - 1),
            )

        def qk_drain(ps, which, hp, nq):
            t_sb, b_sb = (qT_sb, bq_sb) if which == "q" else (kT_sb, bk_sb)
            nc.vector.tensor_scalar_add(
                t_sb[:, hp, 512 * nq : 512 * (nq + 1)], ps[:], b_sb[:, hp : hp + 1]
            )

        def v_mm(ps, st, kc, col0=0, start=None):
            nc.tensor.matmul(
                ps[:, col0 : col0 + M],
                lhsT=xT_sb[:, kc, P * st : P * (st + 1)],
                rhs=wv_sb[:, kc, :],
                start=(kc == 0) if start is None else start,
                stop=(kc == KC - 1),
                skip_group_check=True,
            )

        def transp_tile(hp, jq):
            # SBUF->SBUF transpose on the DMA xbar via the sync ring:
            # no PE, no PSUM bank, no DVE copy
            nc.sync.dma_start(
                hsT_sb[:, hp, jq, :],
                hs_sb[:, jq, P * hp : P * (hp + 1)],
                transpose=True,
            )

        # ---- prefix: 8 tiles accumulate in 8 parallel banks, kc loop
        # outermost so each xT chunk is consumed as it arrives; PE warm-up
        # zero-MMs reuse the same tiles (reset by the kc=0 start=True) ----
        with tc.tile_pool(name="pre_ps", bufs=1, space="PSUM") as pre_ps:
            pre_spec = [("k", 0, 0), ("q", 0, 0), ("q", 0, 1), ("q", 0, 2)]
            pre_qk = [
                pre_ps.tile([P, 512], F32, tag=f"pre{i}", bufs=1, name=f"pre{i}")
                for i in range(len(pre_spec))
            ]
            pre_v = [
                pre_ps.tile([P, M], F32, tag=f"prev{st}", bufs=1, name=f"prev{st}")
                for st in range(4)
            ]
            for i in range(N_WARMUP):
                nc.tensor.matmul(
                    pre_qk[i % len(pre_qk)][:],
                    lhsT=zz_sb[0:1, 0:P],
                    rhs=zz_sb[0:1, 0:512],
                    start=True,
                    stop=True,
                    skip_group_check=True,
                )
            for kc in range(KC):
                for i, (which, hp, nq) in enumerate(pre_spec):
                    qk_mm(pre_qk[i], which, hp, nq, kc)
                for st in range(4):
                    v_mm(pre_v[st], st, kc)
            for i, (which, hp, nq) in enumerate(pre_spec):
                qk_drain(pre_qk[i], which, hp, nq)
            for st in range(4):
                nc.vector.tensor_copy(
                    v_sb[:, st, :, 0:HD],
                    pre_v[st][:].rearrange("p (h d) -> p h d", h=HPC),
                )

        # ---- main pools: hs 3 + sc 4 + fil 1 = 8 banks ----
        with tc.tile_pool(name="attn_ps", bufs=1, space="PSUM") as attn_ps, \
             tc.tile_pool(name="fil_ps", bufs=1, space="PSUM") as fil_ps:

            # filler items: (pe_cost_ns, closure); emitted when afforded
            filler = deque()
            state = {"budget": 0.0}

            def emit_qk(pool, tag, bufs, which, hp, nq):
                ps = pool.tile(
                    [P, 512], F32, tag=tag, bufs=bufs, name=f"p{which}{hp}{nq}"
                )
                for kc in range(KC):
                    qk_mm(ps, which, hp, nq, kc)
                qk_drain(ps, which, hp, nq)

            def emit_vpair(pool, tag, bufs, st):
                ps = pool.tile([P, 512], F32, tag=tag, bufs=bufs, name=f"pv{st}")
                nc.tensor.matmul(
                    ps[:],
                    lhsT=zz_sb[0:1, 0:P],
                    rhs=zz_sb[0:1, 0:512],
                    start=True,
                    stop=True,
                    skip_group_check=True,
                )
                for kc in range(KC):
                    v_mm(ps, st, kc, col0=0, start=False)
                    v_mm(ps, st + 1, kc, col0=M, start=False)
                nc.vector.tensor_copy(
                    v_sb[:, st : st + 2, :, 0:HD],
                    ps[:].rearrange("p (s h d) -> p s h d", s=2, h=HPC),
                )

            def mk_qk(which, hp, nq):
                def fn():
                    emit_qk(fil_ps, "fil", 1, which, hp, nq)
                return QK_COST, fn, ("qk", which, hp, nq)

            def mk_vpair(st):
                def fn():
                    emit_vpair(fil_ps, "fil", 1, st)
                return VPAIR_COST, fn, ("v", st)

            def emit_oproj(pool, tag, bufs, st, dc, copy_eng="v"):
                ps = pool.tile([P, 512], F32, tag=tag, bufs=bufs, name=f"o{st}{dc}")
                for hp in range(2):
                    nc.tensor.matmul(
                        ps[:],
                        lhsT=hsT_sb[:, hp, st, :],
                        rhs=wo_sb[:, hp, 512 * dc : 512 * (dc + 1)],
                        start=(hp == 0),
                        stop=(hp == 1),
                    )
                o_sb = outcp.tile([P, 512], BF16, tag="o", name=f"oc{st}{dc}")
                if copy_eng == "s":
                    nc.scalar.copy(o_sb[:], ps[:])
                else:
                    nc.vector.tensor_copy(o_sb[:], ps[:])
                nc.scalar.dma_start(
                    out_d.ap()[P * st : P * (st + 1), 512 * dc : 512 * (dc + 1)],
                    o_sb[:],
                )

            def mk_oproj(st, dc):
                def fn():
                    emit_oproj(fil_ps, "fil", 1, st, dc)
                return OP_COST, fn, ("op", st, dc)

            def drain_filler():
                while filler and state["budget"] >= filler[0][0]:
                    item = filler.popleft()
                    item[1]()
                    state["budget"] -= item[0]

            def add_budget(ns):
                state["budget"] = min(state["budget"] + ns, BUDGET_CAP)
                drain_filler()

            filler.extend(
                [
                    mk_qk("k", 0, 1),
                    mk_vpair(4),
                    mk_qk("q", 0, 3),
                    mk_vpair(6),
                    mk_qk("k", 1, 0),
                    mk_qk("q", 1, 0),
                    mk_qk("q", 1, 1),
                    mk_qk("k", 0, 2),
                    mk_vpair(8),
                    mk_qk("k", 1, 1),
                    mk_vpair(10),
                    mk_vpair(12),
                    mk_qk("k", 0, 3),
                    mk_vpair(14),
                    mk_qk("q", 1, 2),
                    mk_qk("q", 1, 3),
                    mk_qk("k", 1, 2),
                    mk_qk("k", 1, 3),
                ]
            )

            def attn_phase(hp, ph):
                qlo, qhi = 1024 * ph, 1024 * (ph + 1)
                hs_tiles = [
                    attn_ps.tile([P, 455], F32, tag="hs", bufs=3, name=f"hs{hp}{ph}{i}")
                    for i in range(3)
                ]

                def slot(eta, jql):
                    if jql < 7:
                        return hs_tiles[eta], 65 * jql
                    return hs_tiles[2], 65 * eta

                for t in hs_tiles:
                    nc.tensor.matmul(
                        t[:, 0:455],
                        lhsT=zz_sb[0:1, 0:P],
                        rhs=zz_sb[0:1, 0:455],
                        start=True,
                        stop=True,
                        skip_group_check=True,
                    )

                def emit_scores_exp(kt, q0, w):
                    s_ps = attn_ps.tile(
                        [P, 1024], F32, tag="sc", bufs=2, name=f"sc{hp}{ph}{kt}{q0}"
                    )
                    for eta in range(2):
                        prow = slice(HD * eta, HD * (eta + 1))
                        nc.tensor.matmul(
                            s_ps[:, 512 * eta : 512 * eta + w],
                            lhsT=kT_sb[prow, hp, P * kt : P * (kt + 1)],
                            rhs=qT_sb[prow, hp, q0 : q0 + w],
                            start=True,
                            stop=True,
                        )
                    e_sb = exp_pool.tile([P, 1024], BF16, tag="e", name=f"e{kt}{q0}")
                    pair = s_ps[:].rearrange("p (g f) -> p g f", g=2)[:, :, 0:w]
                    epair = e_sb[:].rearrange("p (g f) -> p g f", g=2)[:, :, 0:w]
                    nc.scalar.activation(epair, pair, EXP, scale=SCALE)
                    if q0 == P * kt:  # chunk starts at the diagonal block
                        nc.vector.tensor_tensor(
                            e_sb[:].rearrange("p (g f) -> p g f", g=2)[:, :, 0:P],
                            e_sb[:].rearrange("p (g f) -> p g f", g=2)[:, :, 0:P],
                            tri_sb[:]
                            .rearrange("p (o f) -> p o f", o=1)
                            .broadcast_to([P, 2, P]),
                            op=mybir.AluOpType.mult,
                        )
                    return e_sb

                def emit_pv(kt, q0, w, e_sb):
                    nblk = 0
                    for eta in range(2):
                        h = 2 * hp + eta
                        for jq in range(q0 // P, (q0 + w) // P):
                            t, col = slot(eta, jq - 8 * ph)
                            nc.tensor.matmul(
                                t[:, col : col + HD + 1],
                                lhsT=e_sb[
                                    :,
                                    512 * eta + P * jq - q0 :
                                    512 * eta + P * jq - q0 + P,
                                ],
                                rhs=v_sb[:, kt, h, :],
                                start=False,
                                stop=(kt == jq),
                                skip_group_check=True,
                            )
                            nblk += 1
                    act_ns = (2 * w + 352) / 1.2 + 120
                    pe_ns = w / 2.4 + nblk * 45
                    add_budget(act_ns - pe_ns)

                def finish_kt(kt):
                    # slot jq=kt complete: normalize eagerly, transpose (DMA)
                    jql = kt - 8 * ph
                    recip_t = recip_pool.tile(
                        [P, 2], F32, tag="re", bufs=8, name=f"re{hp}{ph}{kt}"
                    )
                    for eta in range(2):
                        h = 2 * hp + eta
                        t, col = slot(eta, jql)
                        nc.vector.reciprocal(
                            recip_t[:, eta : eta + 1], t[:, col + HD : col + HD + 1]
                        )
                        nc.vector.tensor_scalar_mul(
                            hs_sb[:, kt, HD * h : HD * (h + 1)],
                            t[:, col : col + HD],
                            recip_t[:, eta : eta + 1],
                        )
                    transp_tile(hp, kt)
                    if hp == 1:
                        filler.append(mk_oproj(kt, 0))
                        filler.append(mk_oproj(kt, 1))

                # chunk list with one-ahead pipelined emission: scores/exp of
                # chunk i+1 are emitted BEFORE PV of chunk i, so the baked PE
                # stream never delays the next exp behind PV + filler work
                chunks = []
                for kt in range(qhi // P):
                    qstart = max(qlo, P * kt)
                    for q0 in range(qstart, qhi, 512):
                        chunks.append((kt, q0, min(512, qhi - q0)))
                prev = None
                for c in chunks:
                    e = emit_scores_exp(*c)
                    if prev is not None:
                        emit_pv(*prev[0], prev[1])
                        if prev[0][0] != c[0] and prev[0][0] >= 8 * ph:
                            finish_kt(prev[0][0])
                    prev = ((c[0], c[1], c[2]), e)
                emit_pv(*prev[0], prev[1])
                if prev[0][0] >= 8 * ph:
                    finish_kt(prev[0][0])

            attn_phase(0, 0)
            attn_phase(0, 1)
            attn_phase(1, 0)
            attn_phase(1, 1)
            tail_specs = [item[2] for item in filler]
            filler.clear()

        # ---- tail: leftovers through 4 parallel banks; ACT is free now so
        # O-proj copies alternate VectorE/ScalarE ----
        if tail_specs:
            with tc.tile_pool(name="tail_ps", bufs=1, space="PSUM") as tail_ps:
                for idx, spec in enumerate(tail_specs):
                    if spec[0] == "op":
                        emit_oproj(
                            tail_ps, "t", 4, spec[1], spec[2],
                            copy_eng="s" if idx % 2 else "v",
                        )
                    elif spec[0] == "qk":
                        emit_qk(tail_ps, "t", 4, spec[1], spec[2], spec[3])
                    else:
                        emit_vpair(tail_ps, "t", 4, spec[1])

    nc.compile()
    return nc


_NC = None


def _get_nc():
    global _NC
    if _NC is None:
        _NC = build_kernel()
    return _NC


def _tri_upper(n=P):
    m = np.zeros((n, n), np.float32)
    iu = np.triu_indices(n, 0)
    m[iu] = 1.0
    return m.astype(ml_dtypes.bfloat16)


def kernel(x, W_Q, W_K, W_V, W_O, b_Q, b_K, b_V, b_O, _trace=False):
    x = np.asarray(x, np.float32)
    W_Q, W_K = np.asarray(W_Q, np.float32), np.asarray(W_K, np.float32)
    W_V, W_O = np.asarray(W_V, np.float32), np.asarray(W_O, np.float32)
    b_Q, b_K = np.asarray(b_Q, np.float32), np.asarray(b_K, np.float32)
    b_V, b_O = np.asarray(b_V, np.float32), np.asarray(b_O, np.float32)

    nc = _get_nc()
    tri = _tri_upper()
    xT_b = [np.ascontiguousarray(x[b].T).astype(ml_dtypes.bfloat16) for b in range(B)]

    def warr(W, cols):  # [D, Mloc] -> [P, KC, Mloc] contiguous
        return np.ascontiguousarray(
            W[:, cols].reshape(KC, P, M).transpose(1, 0, 2)
        ).astype(ml_dtypes.bfloat16)

    in_maps = []
    for core in range(NCORES):
        b, g = core // GROUPS, core % GROUPS
        cols = slice(M * g, M * (g + 1))
        in_maps.append(
            {
                "xT": xT_b[b],
                "wq": warr(W_Q, cols),
                "wk": warr(W_K, cols),
                "wv": warr(W_V, cols),
                "wo": np.ascontiguousarray(
                    W_O[cols, :].reshape(2, P, D).transpose(1, 0, 2)
                ).astype(ml_dtypes.bfloat16),
                "bq": np.ascontiguousarray(b_Q[cols].reshape(2, P).T),
                "bk": np.ascontiguousarray(b_K[cols].reshape(2, P).T),
                "tri": tri,
            }
        )
    res = bass_utils.run_bass_kernel_spmd(
        nc, in_maps, core_ids=list(range(NCORES)), trace=_trace
    )
    const_row = (b_V @ W_O + b_O).astype(np.float32)  # exact: sum(softmax)=1
    out = np.zeros((B, S, D), np.float32)
    for b in range(B):
        acc = res.results[b * GROUPS]["out"].astype(np.float32)
        for g in range(1, GROUPS):
            acc = acc + res.results[b * GROUPS + g]["out"].astype(np.float32)
        out[b] = acc + const_row
    if _trace:
        kernel.last_results = res
    return out


# revision 19
# speedup vs baseline: 1.1313x; 1.1313x over previous
"""Trainium2 Bass kernel for nn_MultiHeadedAttention (B=2, H=16, S=2048, d=64).

Sharding: data-parallel over batch x tensor-parallel over heads.
8 cores = 2 batch groups x 4 head-groups (4 heads each).

v7 (trace-informed rework of the 175us baseline).  The baseline's span was
paced by its single spare PSUM bank: every projection tile, PE transpose and
O-proj tile serialized through one bank at ~2.1us per handoff (~56 handoffs
~= the whole 135us post-startup window).  v7 attacks exactly that:
  - hs->hsT transposes ride the DMA xbar on the sync ring (32 handoffs, all
    PE/PSUM/DVE cost gone); output DMAs move to the scalar ring so the sync
    ring stays xbar-mode-pure.
  - V filler tiles are region-shared two-per-bank (zero-prefill +
    start=False accumulation, the attention-slot trick): 12 -> 6 handoffs.
  - 8 projection tiles (K/Q hp0 nq0-1 + V st0-3) move into a kc-outermost
    prefix that accumulates in 8 parallel banks while xT streams in
    (HBM-bound ~15us), consuming each chunk on arrival.
  - O-proj for the last q-tiles goes to a 4-wide tail pool after the
    attention pools close (copies alternate VectorE/ScalarE - ACT is free).
  - 16 PE warm-up zero-MMs bridge the DMA wait so the HAM clock gate flips
    to 8/8 before real work; a dummy exp preloads the ACT table early.
  - Inputs: host pre-arranges every weight into its contiguous SBUF layout,
    loads split across both HWDGE rings ordered by first use, wo bf16.
  - Output shipped bf16; host sums partials in f32.  In-loop PSUM->SBUF
    copies pinned to VectorE (never ScalarE - the exp stream is critical).
  - All four attention phases normalize eagerly at kt==jq so hsT tiles are
    ready for O-proj as soon as possible.
Attention math is identical to the baseline (see kernel_baseline.py):
fp32->bf16 projections, 2-head row-group-packed score matmuls, one exp per
(kt, 512-chunk) covering both heads (scale=1/8, no max subtraction),
tri-mask on diagonal blocks, PV accumulation with a ones column for the
softmax denominator.  Host adds the exact (b_V @ W_O + b_O) row.
"""

import math
from contextlib import ExitStack

import numpy as np
import ml_dtypes

import concourse.bass as bass
import concourse.mybir as mybir
import concourse.tile as tile
from concourse import bacc, bass_utils

F32 = mybir.dt.float32
BF16 = mybir.dt.bfloat16
EXP = mybir.ActivationFunctionType.Exp

B, S, D = 2, 2048, 1024
NH, HD = 16, 64
NCORES = 8
GROUPS = NCORES // B          # 4 head-groups per batch
HPC = NH // GROUPS            # 4 heads per core
M = HPC * HD                  # 256 local head-dims per core
P = 128
KC = D // P                   # 8 contraction chunks
NT = S // P                   # 16 q/s tiles
SCALE = 1.0 / math.sqrt(HD)   # 0.125
N_WARMUP = 16
TAIL_ST = 10                  # O-proj for st >= TAIL_ST goes to the tail pool


def build_kernel():
    nc = bacc.Bacc("TRN2", target_bir_lowering=False)

    xT_d = nc.dram_tensor("xT", [D, S], BF16, kind="ExternalInput")
    wq_d = nc.dram_tensor("wq", [P, KC, M], BF16, kind="ExternalInput")
    wk_d = nc.dram_tensor("wk", [P, KC, M], BF16, kind="ExternalInput")
    wv_d = nc.dram_tensor("wv", [P, KC, M], BF16, kind="ExternalInput")
    wo_d = nc.dram_tensor("wo", [P, 2, D], BF16, kind="ExternalInput")
    bq_d = nc.dram_tensor("bq", [P, 2], F32, kind="ExternalInput")
    bk_d = nc.dram_tensor("bk", [P, 2], F32, kind="ExternalInput")
    tri_d = nc.dram_tensor("tri", [P, P], BF16, kind="ExternalInput")
    out_d = nc.dram_tensor("out", [S, D], BF16, kind="ExternalOutput")

    with tile.TileContext(nc) as tc, ExitStack() as ctx:
        big = ctx.enter_context(tc.tile_pool(name="big", bufs=1))
        exp_pool = ctx.enter_context(tc.tile_pool(name="expp", bufs=8))
        outcp = ctx.enter_context(tc.tile_pool(name="outcp", bufs=4))
        recip_pool = ctx.enter_context(tc.tile_pool(name="recipp", bufs=2))

        # ---- persistent SBUF tiles ----
        xT_sb = big.tile([P, KC, S], BF16)
        wq_sb = big.tile([P, KC, M], BF16)
        wk_sb = big.tile([P, KC, M], BF16)
        wv_sb = big.tile([P, KC, M], BF16)
        wo_sb = big.tile([P, 2, D], BF16)
        bq_sb = big.tile([P, 2], F32)
        bk_sb = big.tile([P, 2], F32)
        qT_sb = big.tile([P, 2, S], BF16)
        kT_sb = big.tile([P, 2, S], BF16)
        v_sb = big.tile([P, NT, HPC, HD + 1], BF16)
        hs_sb = big.tile([P, NT, M], BF16)
        hsT_sb = big.tile([P, 2, NT, P], BF16)
        tri_sb = big.tile([P, P], BF16)
        zz_sb = big.tile([1, 512], BF16)
        dummy_sb = big.tile([1, 2], BF16)

        nc.vector.memset(zz_sb[:], 0.0)
        nc.vector.memset(v_sb[:, :, :, HD : HD + 1], 1.0)
        # preload the ACT exp table (~2.7us) during the DMA prefix
        nc.scalar.activation(dummy_sb[:], zz_sb[0:1, 0:2], EXP, scale=SCALE)

        # ---- input DMAs: contiguous slabs on both rings, by first use ----
        nc.sync.dma_start(wq_sb[:], wq_d.ap())
        nc.scalar.dma_start(wk_sb[:], wk_d.ap())
        nc.scalar.dma_start(bk_sb[:], bk_d.ap())
        nc.scalar.dma_start(bq_sb[:], bq_d.ap())
        nc.scalar.dma_start(wv_sb[:], wv_d.ap())
        for kc in range(KC):
            eng = nc.sync if kc % 2 == 0 else nc.scalar
            eng.dma_start(xT_sb[:, kc, :], xT_d.ap()[P * kc : P * (kc + 1), :])
        nc.sync.dma_start(tri_sb[:], tri_d.ap())
        nc.scalar.dma_start(wo_sb[:], wo_d.ap())

        # ---- building blocks ----
        def qk_mm(ps, which, hp, nq, kc):
            w_sb = wq_sb if which == "q" else wk_sb
            nc.tensor.matmul(
                ps[:],
                lhsT=w_sb[:, kc, P * hp : P * (hp + 1)],
                rhs=xT_sb[:, kc, 512 * nq : 512 * (nq + 1)],
                start=(kc == 0),
                stop=(kc == KC - 1),
            )

        def qk_drain(ps, which, hp, nq):
            t_sb, b_sb = (qT_sb, bq_sb) if which == "q" else (kT_sb, bk_sb)
            nc.vector.tensor_scalar_add(
                t_sb[:, hp, 512 * nq : 512 * (nq + 1)], ps[:], b_sb[:, hp : hp + 1]
            )

        def v_mm(ps, st, kc, col0=0, start=None):
            nc.tensor.matmul(
                ps[:, col0 : col0 + M],
                lhsT=xT_sb[:, kc, P * st : P * (st + 1)],
                rhs=wv_sb[:, kc, :],
                start=(kc == 0) if start is None else start,
                stop=(kc == KC - 1),
                skip_group_check=True,
            )

        def transp_tile(hp, jq):
            # SBUF->SBUF transpose on the DMA xbar via the sync ring:
            # no PE, no PSUM bank, no DVE copy
            nc.sync.dma_start(
                hsT_sb[:, hp, jq, :],
                hs_sb[:, jq, P * hp : P * (hp + 1)],
                transpose=True,
            )

        # ---- prefix: 8 tiles accumulate in 8 parallel banks, kc loop
        # outermost so each xT chunk is consumed as it arrives; PE warm-up
        # zero-MMs reuse the same tiles (reset by the kc=0 start=True) ----
        with tc.tile_pool(name="pre_ps", bufs=1, space="PSUM") as pre_ps:
            pre_spec = [("k", 0, 0), ("q", 0, 0), ("q", 0, 1), ("k", 0, 1)]
            pre_qk = [
                pre_ps.tile([P, 512], F32, tag=f"pre{i}", bufs=1, name=f"pre{i}")
                for i in range(len(pre_spec))
            ]
            pre_v = [
                pre_ps.tile([P, M], F32, tag=f"prev{st}", bufs=1, name=f"prev{st}")
                for st in range(4)
            ]
            for i in range(N_WARMUP):
                nc.tensor.matmul(
                    pre_qk[i % len(pre_qk)][:],
                    lhsT=zz_sb[0:1, 0:P],
                    rhs=zz_sb[0:1, 0:512],
                    start=True,
                    stop=True,
                    skip_group_check=True,
                )
            for kc in range(KC):
                for i, (which, hp, nq) in enumerate(pre_spec):
                    qk_mm(pre_qk[i], which, hp, nq, kc)
                for st in range(4):
                    v_mm(pre_v[st], st, kc)
            for i, (which, hp, nq) in enumerate(pre_spec):
                qk_drain(pre_qk[i], which, hp, nq)
            for st in range(4):
                nc.vector.tensor_copy(
                    v_sb[:, st, :, 0:HD],
                    pre_v[st][:].rearrange("p (h d) -> p h d", h=HPC),
                )

        # ---- main pools: hs 3 + sc 4 + fil 1 = 8 banks ----
        with tc.tile_pool(name="attn_ps", bufs=1, space="PSUM") as attn_ps, \
             tc.tile_pool(name="fil_ps", bufs=1, space="PSUM") as fil_ps:

            def emit_qk(pool, tag, bufs, which, hp, nq):
                ps = pool.tile(
                    [P, 512], F32, tag=tag, bufs=bufs, name=f"p{which}{hp}{nq}"
                )
                for kc in range(KC):
                    qk_mm(ps, which, hp, nq, kc)
                qk_drain(ps, which, hp, nq)

            def emit_vpair(pool, tag, bufs, st):
                ps = pool.tile([P, 512], F32, tag=tag, bufs=bufs, name=f"pv{st}")
                nc.tensor.matmul(
                    ps[:],
                    lhsT=zz_sb[0:1, 0:P],
                    rhs=zz_sb[0:1, 0:512],
                    start=True,
                    stop=True,
                    skip_group_check=True,
                )
                for kc in range(KC):
                    v_mm(ps, st, kc, col0=0, start=False)
                    v_mm(ps, st + 1, kc, col0=M, start=False)
                nc.vector.tensor_copy(
                    v_sb[:, st : st + 2, :, 0:HD],
                    ps[:].rearrange("p (s h d) -> p s h d", s=2, h=HPC),
                )

            def emit_oproj(pool, tag, bufs, st, dc, copy_eng="v"):
                ps = pool.tile([P, 512], F32, tag=tag, bufs=bufs, name=f"o{st}{dc}")
                for hp in range(2):
                    nc.tensor.matmul(
                        ps[:],
                        lhsT=hsT_sb[:, hp, st, :],
                        rhs=wo_sb[:, hp, 512 * dc : 512 * (dc + 1)],
                        start=(hp == 0),
                        stop=(hp == 1),
                    )
                o_sb = outcp.tile([P, 512], BF16, tag="o", name=f"oc{st}{dc}")
                if copy_eng == "s":
                    nc.scalar.copy(o_sb[:], ps[:])
                else:
                    nc.vector.tensor_copy(o_sb[:], ps[:])
                nc.scalar.dma_start(
                    out_d.ap()[P * st : P * (st + 1), 512 * dc : 512 * (dc + 1)],
                    o_sb[:],
                )

            def attn_phase(hp, ph):
                qlo, qhi = 1024 * ph, 1024 * (ph + 1)
                hs_tiles = [
                    attn_ps.tile([P, 455], F32, tag="hs", bufs=3, name=f"hs{hp}{ph}{i}")
                    for i in range(3)
                ]

                def slot(eta, jql):
                    if jql < 7:
                        return hs_tiles[eta], 65 * jql
                    return hs_tiles[2], 65 * eta

                for t in hs_tiles:
                    nc.tensor.matmul(
                        t[:, 0:455],
                        lhsT=zz_sb[0:1, 0:P],
                        rhs=zz_sb[0:1, 0:455],
                        start=True,
                        stop=True,
                        skip_group_check=True,
                    )
                for kt in range(qhi // P):
                    qstart = max(qlo, P * kt)
                    for q0 in range(qstart, qhi, 512):
                        w = min(512, qhi - q0)
                        s_ps = attn_ps.tile(
                            [P, 1024], F32, tag="sc", bufs=2, name=f"sc{hp}{ph}{kt}{q0}"
                        )
                        for eta in range(2):
                            prow = slice(HD * eta, HD * (eta + 1))
                            nc.tensor.matmul(
                                s_ps[:, 512 * eta : 512 * eta + w],
                                lhsT=kT_sb[prow, hp, P * kt : P * (kt + 1)],
                                rhs=qT_sb[prow, hp, q0 : q0 + w],
                                start=True,
                                stop=True,
                            )
                        e_sb = exp_pool.tile(
                            [P, 1024], BF16, tag="e", name=f"e{kt}{q0}"
                        )
                        pair = s_ps[:].rearrange("p (g f) -> p g f", g=2)[:, :, 0:w]
                        epair = e_sb[:].rearrange("p (g f) -> p g f", g=2)[:, :, 0:w]
                        nc.scalar.activation(epair, pair, EXP, scale=SCALE)
                        if q0 == P * kt:  # chunk starts at the diagonal block
                            nc.vector.tensor_tensor(
                                e_sb[:].rearrange("p (g f) -> p g f", g=2)[:, :, 0:P],
                                e_sb[:].rearrange("p (g f) -> p g f", g=2)[:, :, 0:P],
                                tri_sb[:]
                                .rearrange("p (o f) -> p o f", o=1)
                                .broadcast_to([P, 2, P]),
                                op=mybir.AluOpType.mult,
                            )
                        for eta in range(2):
                            h = 2 * hp + eta
                            for jq in range(q0 // P, (q0 + w) // P):
                                t, col = slot(eta, jq - 8 * ph)
                                nc.tensor.matmul(
                                    t[:, col : col + HD + 1],
                                    lhsT=e_sb[
                                        :,
                                        512 * eta + P * jq - q0 :
                                        512 * eta + P * jq - q0 + P,
                                    ],
                                    rhs=v_sb[:, kt, h, :],
                                    start=False,
                                    stop=(kt == jq),
                                    skip_group_check=True,
                                )
                    if kt >= 8 * ph:
                        # slot jq=kt complete: normalize eagerly + transpose
                        jql = kt - 8 * ph
                        recip_t = recip_pool.tile(
                            [P, 2], F32, tag="re", bufs=8, name=f"re{hp}{ph}{kt}"
                        )
                        for eta in range(2):
                            h = 2 * hp + eta
                            t, col = slot(eta, jql)
                            nc.vector.reciprocal(
                                recip_t[:, eta : eta + 1],
                                t[:, col + HD : col + HD + 1],
                            )
                            nc.vector.tensor_scalar_mul(
                                hs_sb[:, kt, HD * h : HD * (h + 1)],
                                t[:, col : col + HD],
                                recip_t[:, eta : eta + 1],
                            )
                        transp_tile(hp, kt)
                        if hp == 1 and kt < TAIL_ST:
                            emit_oproj(fil_ps, "fil", 1, kt, 0)
                            emit_oproj(fil_ps, "fil", 1, kt, 1)

            # filler for attn(0,0): V st4-7 (PV kt 4-7)
            emit_vpair(fil_ps, "fil", 1, 4)
            emit_vpair(fil_ps, "fil", 1, 6)
            attn_phase(0, 0)
            # filler for attn(0,1): remaining hp0 q/k + V st8-15
            emit_qk(fil_ps, "fil", 1, "q", 0, 2)
            emit_qk(fil_ps, "fil", 1, "q", 0, 3)
            emit_vpair(fil_ps, "fil", 1, 8)
            emit_qk(fil_ps, "fil", 1, "k", 0, 2)
            emit_vpair(fil_ps, "fil", 1, 10)
            emit_qk(fil_ps, "fil", 1, "k", 0, 3)
            emit_vpair(fil_ps, "fil", 1, 12)
            emit_vpair(fil_ps, "fil", 1, 14)
            attn_phase(0, 1)
            # filler for attn(1,*): head-pair 1 projections
            for which, hp, nq in [
                ("k", 1, 0), ("q", 1, 0), ("q", 1, 1), ("k", 1, 1),
                ("q", 1, 2), ("q", 1, 3), ("k", 1, 2), ("k", 1, 3),
            ]:
                emit_qk(fil_ps, "fil", 1, which, hp, nq)
            attn_phase(1, 0)
            attn_phase(1, 1)

        # ---- tail: O-proj for the last q-tiles through 4 parallel banks;
        # ACT is free now so copies alternate VectorE/ScalarE ----
        with tc.tile_pool(name="tail_ps", bufs=1, space="PSUM") as tail_ps:
            idx = 0
            for st in range(TAIL_ST, NT):
                for dc in range(2):
                    emit_oproj(
                        tail_ps, "t", 4, st, dc, copy_eng="s" if idx % 2 else "v"
                    )
                    idx += 1

    nc.compile()
    return nc


_NC = None


def _get_nc():
    global _NC
    if _NC is None:
        _NC = build_kernel()
    return _NC


def _tri_upper(n=P):
    m = np.zeros((n, n), np.float32)
    iu = np.triu_indices(n, 0)
    m[iu] = 1.0
    return m.astype(ml_dtypes.bfloat16)


def kernel(x, W_Q, W_K, W_V, W_O, b_Q, b_K, b_V, b_O, _trace=False):
    x = np.asarray(x, np.float32)
    W_Q, W_K = np.asarray(W_Q, np.float32), np.asarray(W_K, np.float32)
    W_V, W_O = np.asarray(W_V, np.float32), np.asarray(W_O, np.float32)
    b_Q, b_K = np.asarray(b_Q, np.float32), np.asarray(b_K, np.float32)
    b_V, b_O = np.asarray(b_V, np.float32), np.asarray(b_O, np.float32)

    nc = _get_nc()
    tri = _tri_upper()
    xT_b = [np.ascontiguousarray(x[b].T).astype(ml_dtypes.bfloat16) for b in range(B)]

    def warr(W, cols):  # [D, Mloc] -> [P, KC, Mloc] contiguous
        return np.ascontiguousarray(
            W[:, cols].reshape(KC, P, M).transpose(1, 0, 2)
        ).astype(ml_dtypes.bfloat16)

    in_maps = []
    for core in range(NCORES):
        b, g = core // GROUPS, core % GROUPS
        cols = slice(M * g, M * (g + 1))
        in_maps.append(
            {
                "xT": xT_b[b],
                "wq": warr(W_Q, cols),
                "wk": warr(W_K, cols),
                "wv": warr(W_V, cols),
                "wo": np.ascontiguousarray(
                    W_O[cols, :].reshape(2, P, D).transpose(1, 0, 2)
                ).astype(ml_dtypes.bfloat16),
                "bq": np.ascontiguousarray(b_Q[cols].reshape(2, P).T),
                "bk": np.ascontiguousarray(b_K[cols].reshape(2, P).T),
                "tri": tri,
            }
        )
    res = bass_utils.run_bass_kernel_spmd(
        nc, in_maps, core_ids=list(range(NCORES)), trace=_trace
    )
    const_row = (b_V @ W_O + b_O).astype(np.float32)  # exact: sum(softmax)=1
    out = np.zeros((B, S, D), np.float32)
    for b in range(B):
        acc = res.results[b * GROUPS]["out"].astype(np.float32)
        for g in range(1, GROUPS):
            acc = acc + res.results[b * GROUPS + g]["out"].astype(np.float32)
        out[b] = acc + const_row
    if _trace:
        kernel.last_results = res
    return out


# revision 20
# speedup vs baseline: 1.1361x; 1.0042x over previous
"""Trainium2 Bass kernel for nn_MultiHeadedAttention (B=2, H=16, S=2048, d=64).

Sharding: data-parallel over batch x tensor-parallel over heads.
8 cores = 2 batch groups x 4 head-groups (4 heads each).

v8 (trace-informed rework of the 175us baseline):
  - Inputs (5.6MB, HBM-bound ~16us) in SEVEN big dma_starts instead of ~16
    small ones (per-dma_start serialization cost ~1-2us each on a ring):
    sync ring carries xT in four 2-chunk slabs, scalar ring carries biases,
    a combined wq|wk|wv slab and a combined wo|tri slab.  Everything is
    host-pre-arranged to be a single contiguous (or regularly strided)
    transfer in the exact SBUF layout.
  - Prefix: 8 projection tiles (K/Q hp0 nq0-1 + V st0-3) accumulate in 8
    parallel PSUM banks with the kc loop outermost, consuming each xT slab
    on arrival.  First scores fire ~1us after the last slab.
  - Real PE warm-up: 8 full-array (128-contraction) matmuls on a zeroed
    SBUF tile bridge the DMA wait so the HAM clock gate reaches 8/8 before
    the prefix (1-row zero matmuls do NOT register as PE activity).
  - The attention inner loop is ACT-bound, so filler work (remaining QKV
    projections, O-proj) is interleaved at per-kt granularity via an
    explicit map: each item lands a few kt-groups before the attention
    that consumes it, so the single spare PSUM bank never bunches and the
    exp stream never starves.  V tiles are region-shared two-per-bank.
  - hs->hsT transposes ride the DMA xbar on the sync ring (no PE, no PSUM,
    no DVE); output DMAs go on the scalar ring (keeps sync xbar-mode-pure).
  - O-proj for the last q-tiles drains through a 4-wide tail pool after the
    attention pools close, copies alternating VectorE/ScalarE.
  - Output shipped bf16; in-loop PSUM->SBUF copies pinned to VectorE.
Attention math is identical to the baseline (see kernel_baseline.py):
fp32->bf16 projections, 2-head row-group-packed score matmuls, one exp per
(kt, 512-chunk) covering both heads (scale=1/8, no max subtraction),
tri-mask on diagonal blocks, PV accumulation with a ones column for the
softmax denominator, eager normalization at kt==jq.  Host adds the exact
(b_V @ W_O + b_O) row.
"""

import math
from contextlib import ExitStack

import numpy as np
import ml_dtypes

import concourse.bass as bass
import concourse.mybir as mybir
import concourse.tile as tile
from concourse import bacc, bass_utils

F32 = mybir.dt.float32
BF16 = mybir.dt.bfloat16
EXP = mybir.ActivationFunctionType.Exp

B, S, D = 2, 2048, 1024
NH, HD = 16, 64
NCORES = 8
GROUPS = NCORES // B          # 4 head-groups per batch
HPC = NH // GROUPS            # 4 heads per core
M = HPC * HD                  # 256 local head-dims per core
M3 = 3 * M
P = 128
KC = D // P                   # 8 contraction chunks
NT = S // P                   # 16 q/s tiles
SCALE = 1.0 / math.sqrt(HD)   # 0.125
N_WARMUP = 8
TAIL_ST = 10                  # O-proj for st >= TAIL_ST goes to the tail pool
WT = 2 * D + P                # wo|tri combo free size


def build_kernel():
    nc = bacc.Bacc("TRN2", target_bir_lowering=False)

    xT_d = nc.dram_tensor("xT", [D, S], BF16, kind="ExternalInput")
    wqkv_d = nc.dram_tensor("wqkv", [P, KC, M3], BF16, kind="ExternalInput")
    wt_d = nc.dram_tensor("wt", [P, WT], BF16, kind="ExternalInput")
    bqk_d = nc.dram_tensor("bqk", [P, 4], F32, kind="ExternalInput")
    out_d = nc.dram_tensor("out", [S, D], BF16, kind="ExternalOutput")

    with tile.TileContext(nc) as tc, ExitStack() as ctx:
        big = ctx.enter_context(tc.tile_pool(name="big", bufs=1))
        exp_pool = ctx.enter_context(tc.tile_pool(name="expp", bufs=8))
        outcp = ctx.enter_context(tc.tile_pool(name="outcp", bufs=4))
        recip_pool = ctx.enter_context(tc.tile_pool(name="recipp", bufs=2))

        # ---- persistent SBUF tiles ----
        xT_sb = big.tile([P, KC, S], BF16)
        wqkv_sb = big.tile([P, KC, M3], BF16)
        wt_sb = big.tile([P, WT], BF16)
        bqk_sb = big.tile([P, 4], F32)
        qT_sb = big.tile([P, 2, S], BF16)
        kT_sb = big.tile([P, 2, S], BF16)
        v_sb = big.tile([P, NT, HPC, HD + 1], BF16)
        hs_sb = big.tile([P, NT, M], BF16)
        hsT_sb = big.tile([P, 2, NT, P], BF16)
        zz_sb = big.tile([1, 512], BF16)
        wu_sb = big.tile([P, 512], BF16)
        dummy_sb = big.tile([1, 2], BF16)

        def wq_ap(kc, hp):
            return wqkv_sb[:, kc, P * hp : P * (hp + 1)]

        def wk_ap(kc, hp):
            return wqkv_sb[:, kc, M + P * hp : M + P * (hp + 1)]

        def wv_ap(kc):
            return wqkv_sb[:, kc, 2 * M : 3 * M]

        def wo_ap(hp, dc):
            return wt_sb[:, D * hp + 512 * dc : D * hp + 512 * (dc + 1)]

        tri_ap = wt_sb[:, 2 * D : 2 * D + P]

        nc.vector.memset(zz_sb[:], 0.0)
        nc.vector.memset(wu_sb[:], 0.0)
        nc.vector.memset(v_sb[:, :, :, HD : HD + 1], 1.0)
        # preload the ACT exp table (~2.7us) during the DMA prefix
        nc.scalar.activation(dummy_sb[:], zz_sb[0:1, 0:2], EXP, scale=SCALE)

        # ---- input DMAs: few big transfers, both rings, by first use ----
        nc.scalar.dma_start(bqk_sb[:], bqk_d.ap())
        nc.scalar.dma_start(wqkv_sb[:], wqkv_d.ap())
        for i in range(4):
            nc.sync.dma_start(
                xT_sb[:, 2 * i : 2 * i + 2, :],
                xT_d.ap()[256 * i : 256 * (i + 1), :].rearrange(
                    "(c p) s -> p c s", p=P
                ),
            )
        nc.scalar.dma_start(wt_sb[:], wt_d.ap())

        # ---- building blocks ----
        def qk_mm(ps, which, hp, nq, kc):
            w = wq_ap(kc, hp) if which == "q" else wk_ap(kc, hp)
            nc.tensor.matmul(
                ps[:],
                lhsT=w,
                rhs=xT_sb[:, kc, 512 * nq : 512 * (nq + 1)],
                start=(kc == 0),
                stop=(kc == KC - 1),
            )

        def qk_drain(ps, which, hp, nq):
            t_sb = qT_sb if which == "q" else kT_sb
            boff = hp if which == "q" else 2 + hp
            nc.vector.tensor_scalar_add(
                t_sb[:, hp, 512 * nq : 512 * (nq + 1)],
                ps[:],
                bqk_sb[:, boff : boff + 1],
            )

        def v_mm(ps, st, kc, col0=0, start=None):
            nc.tensor.matmul(
                ps[:, col0 : col0 + M],
                lhsT=xT_sb[:, kc, P * st : P * (st + 1)],
                rhs=wv_ap(kc),
                start=(kc == 0) if start is None else start,
                stop=(kc == KC - 1),
                skip_group_check=True,
            )

        def transp_tile(hp, jq):
            # SBUF->SBUF transpose on the DMA xbar via the sync ring
            nc.sync.dma_start(
                hsT_sb[:, hp, jq, :],
                hs_sb[:, jq, P * hp : P * (hp + 1)],
                transpose=True,
            )

        # ---- prefix: 8 tiles accumulate in 8 parallel banks, kc loop
        # outermost; real full-array warm-up matmuls bridge the DMA wait ----
        with tc.tile_pool(name="pre_ps", bufs=1, space="PSUM") as pre_ps:
            pre_spec = [("k", 0, 0), ("q", 0, 0), ("q", 0, 1), ("k", 0, 1)]
            pre_qk = [
                pre_ps.tile([P, 512], F32, tag=f"pre{i}", bufs=1, name=f"pre{i}")
                for i in range(len(pre_spec))
            ]
            pre_v = [
                pre_ps.tile([P, M], F32, tag=f"prev{st}", bufs=1, name=f"prev{st}")
                for st in range(4)
            ]
            for i in range(N_WARMUP):
                nc.tensor.matmul(
                    pre_qk[i % len(pre_qk)][:],
                    lhsT=wu_sb[:, 0:P],
                    rhs=wu_sb[:],
                    start=True,
                    stop=True,
                    skip_group_check=True,
                )
            for kc in range(KC):
                for i, (which, hp, nq) in enumerate(pre_spec):
                    qk_mm(pre_qk[i], which, hp, nq, kc)
                for st in range(4):
                    v_mm(pre_v[st], st, kc)
            for i, (which, hp, nq) in enumerate(pre_spec):
                qk_drain(pre_qk[i], which, hp, nq)
            for st in range(4):
                nc.vector.tensor_copy(
                    v_sb[:, st, :, 0:HD],
                    pre_v[st][:].rearrange("p (h d) -> p h d", h=HPC),
                )

        # ---- main pools: hs 3 + sc 4 + fil 1 = 8 banks ----
        with tc.tile_pool(name="attn_ps", bufs=1, space="PSUM") as attn_ps, \
             tc.tile_pool(name="fil_ps", bufs=1, space="PSUM") as fil_ps:

            def emit_qk(pool, tag, bufs, which, hp, nq):
                ps = pool.tile(
                    [P, 512], F32, tag=tag, bufs=bufs, name=f"p{which}{hp}{nq}"
                )
                for kc in range(KC):
                    qk_mm(ps, which, hp, nq, kc)
                qk_drain(ps, which, hp, nq)

            def emit_vpair(pool, tag, bufs, st):
                ps = pool.tile([P, 512], F32, tag=tag, bufs=bufs, name=f"pv{st}")
                nc.tensor.matmul(
                    ps[:],
                    lhsT=zz_sb[0:1, 0:P],
                    rhs=zz_sb[0:1, 0:512],
                    start=True,
                    stop=True,
                    skip_group_check=True,
                )
                for kc in range(KC):
                    v_mm(ps, st, kc, col0=0, start=False)
                    v_mm(ps, st + 1, kc, col0=M, start=False)
                nc.vector.tensor_copy(
                    v_sb[:, st : st + 2, :, 0:HD],
                    ps[:].rearrange("p (s h d) -> p s h d", s=2, h=HPC),
                )

            def emit_oproj(pool, tag, bufs, st, dc, copy_eng="v"):
                ps = pool.tile([P, 512], F32, tag=tag, bufs=bufs, name=f"o{st}{dc}")
                for hp in range(2):
                    nc.tensor.matmul(
                        ps[:],
                        lhsT=hsT_sb[:, hp, st, :],
                        rhs=wo_ap(hp, dc),
                        start=(hp == 0),
                        stop=(hp == 1),
                    )
                o_sb = outcp.tile([P, 512], BF16, tag="o", name=f"oc{st}{dc}")
                if copy_eng == "s":
                    nc.scalar.copy(o_sb[:], ps[:])
                else:
                    nc.vector.tensor_copy(o_sb[:], ps[:])
                nc.scalar.dma_start(
                    out_d.ap()[P * st : P * (st + 1), 512 * dc : 512 * (dc + 1)],
                    o_sb[:],
                )

            def qk(which, hp, nq):
                return lambda: emit_qk(fil_ps, "fil", 1, which, hp, nq)

            def vp(st):
                return lambda: emit_vpair(fil_ps, "fil", 1, st)

            def attn_phase(hp, ph, post_kt):
                qlo, qhi = 1024 * ph, 1024 * (ph + 1)
                hs_tiles = [
                    attn_ps.tile([P, 455], F32, tag="hs", bufs=3, name=f"hs{hp}{ph}{i}")
                    for i in range(3)
                ]

                def slot(eta, jql):
                    if jql < 7:
                        return hs_tiles[eta], 65 * jql
                    return hs_tiles[2], 65 * eta

                for t in hs_tiles:
                    nc.tensor.matmul(
                        t[:, 0:455],
                        lhsT=zz_sb[0:1, 0:P],
                        rhs=zz_sb[0:1, 0:455],
                        start=True,
                        stop=True,
                        skip_group_check=True,
                    )
                for kt in range(qhi // P):
                    qstart = max(qlo, P * kt)
                    for q0 in range(qstart, qhi, 512):
                        w = min(512, qhi - q0)
                        s_ps = attn_ps.tile(
                            [P, 1024], F32, tag="sc", bufs=2, name=f"sc{hp}{ph}{kt}{q0}"
                        )
                        for eta in range(2):
                            prow = slice(HD * eta, HD * (eta + 1))
                            nc.tensor.matmul(
                                s_ps[:, 512 * eta : 512 * eta + w],
                                lhsT=kT_sb[prow, hp, P * kt : P * (kt + 1)],
                                rhs=qT_sb[prow, hp, q0 : q0 + w],
                                start=True,
                                stop=True,
                            )
                        e_sb = exp_pool.tile(
                            [P, 1024], BF16, tag="e", name=f"e{kt}{q0}"
                        )
                        pair = s_ps[:].rearrange("p (g f) -> p g f", g=2)[:, :, 0:w]
                        epair = e_sb[:].rearrange("p (g f) -> p g f", g=2)[:, :, 0:w]
                        nc.scalar.activation(epair, pair, EXP, scale=SCALE)
                        if q0 == P * kt:  # chunk starts at the diagonal block
                            nc.vector.tensor_tensor(
                                e_sb[:].rearrange("p (g f) -> p g f", g=2)[:, :, 0:P],
                                e_sb[:].rearrange("p (g f) -> p g f", g=2)[:, :, 0:P],
                                tri_ap.rearrange("p (o f) -> p o f", o=1)
                                .broadcast_to([P, 2, P]),
                                op=mybir.AluOpType.mult,
                            )
                        for eta in range(2):
                            h = 2 * hp + eta
                            for jq in range(q0 // P, (q0 + w) // P):
                                t, col = slot(eta, jq - 8 * ph)
                                nc.tensor.matmul(
                                    t[:, col : col + HD + 1],
                                    lhsT=e_sb[
                                        :,
                                        512 * eta + P * jq - q0 :
                                        512 * eta + P * jq - q0 + P,
                                    ],
                                    rhs=v_sb[:, kt, h, :],
                                    start=False,
                                    stop=(kt == jq),
                                    skip_group_check=True,
                                )
                    if kt >= 8 * ph:
                        # slot jq=kt complete: normalize eagerly + transpose
                        jql = kt - 8 * ph
                        recip_t = recip_pool.tile(
                            [P, 2], F32, tag="re", bufs=8, name=f"re{hp}{ph}{kt}"
                        )
                        for eta in range(2):
                            h = 2 * hp + eta
                            t, col = slot(eta, jql)
                            nc.vector.reciprocal(
                                recip_t[:, eta : eta + 1],
                                t[:, col + HD : col + HD + 1],
                            )
                            nc.vector.tensor_scalar_mul(
                                hs_sb[:, kt, HD * h : HD * (h + 1)],
                                t[:, col : col + HD],
                                recip_t[:, eta : eta + 1],
                            )
                        transp_tile(hp, kt)
                        if hp == 1 and kt < TAIL_ST:
                            emit_oproj(fil_ps, "fil", 1, kt, 0)
                            emit_oproj(fil_ps, "fil", 1, kt, 1)
                    for fn in post_kt.get(kt, []):
                        fn()

            attn_phase(0, 0, {1: [vp(4)], 2: [vp(6)], 4: [qk("q", 0, 2)],
                              5: [qk("q", 0, 3)]})
            attn_phase(0, 1, {0: [vp(8)], 1: [qk("k", 0, 2)], 2: [vp(10)],
                              3: [qk("k", 1, 0)], 4: [qk("q", 1, 0)],
                              5: [qk("q", 1, 1)], 6: [qk("k", 1, 1)],
                              7: [qk("k", 0, 3)], 8: [vp(12)], 9: [vp(14)],
                              10: [qk("q", 1, 2)], 11: [qk("q", 1, 3)],
                              12: [qk("k", 1, 2)], 13: [qk("k", 1, 3)]})
            attn_phase(1, 0, {})
            attn_phase(1, 1, {})

        # ---- tail: O-proj for the last q-tiles through 4 parallel banks;
        # ACT is free now so copies alternate VectorE/ScalarE ----
        with tc.tile_pool(name="tail_ps", bufs=1, space="PSUM") as tail_ps:
            idx = 0
            for st in range(TAIL_ST, NT):
                for dc in range(2):
                    emit_oproj(
                        tail_ps, "t", 4, st, dc, copy_eng="s" if idx % 2 else "v"
                    )
                    idx += 1

    nc.compile()
    return nc


_NC = None


def _get_nc():
    global _NC
    if _NC is None:
        _NC = build_kernel()
    return _NC


def _tri_upper(n=P):
    m = np.zeros((n, n), np.float32)
    iu = np.triu_indices(n, 0)
    m[iu] = 1.0
    return m


def kernel(x, W_Q, W_K, W_V, W_O, b_Q, b_K, b_V, b_O, _trace=False):
    x = np.asarray(x, np.float32)
    W_Q, W_K = np.asarray(W_Q, np.float32), np.asarray(W_K, np.float32)
    W_V, W_O = np.asarray(W_V, np.float32), np.asarray(W_O, np.float32)
    b_Q, b_K = np.asarray(b_Q, np.float32), np.asarray(b_K, np.float32)
    b_V, b_O = np.asarray(b_V, np.float32), np.asarray(b_O, np.float32)

    nc = _get_nc()
    xT_b = [np.ascontiguousarray(x[b].T).astype(ml_dtypes.bfloat16) for b in range(B)]

    def warr(W, cols):  # [D, Mloc] -> [P, KC, Mloc]
        return W[:, cols].reshape(KC, P, M).transpose(1, 0, 2)

    tri = _tri_upper()
    in_maps = []
    for core in range(NCORES):
        b, g = core // GROUPS, core % GROUPS
        cols = slice(M * g, M * (g + 1))
        wqkv = np.concatenate(
            [warr(W_Q, cols), warr(W_K, cols), warr(W_V, cols)], axis=2
        )
        wo = W_O[cols, :].reshape(2, P, D).transpose(1, 0, 2).reshape(P, 2 * D)
        wt = np.concatenate([wo, tri], axis=1)
        bqk = np.stack(
            [
                b_Q[cols][0:P], b_Q[cols][P : 2 * P],
                b_K[cols][0:P], b_K[cols][P : 2 * P],
            ],
            axis=1,
        )
        in_maps.append(
            {
                "xT": xT_b[b],
                "wqkv": np.ascontiguousarray(wqkv).astype(ml_dtypes.bfloat16),
                "wt": np.ascontiguousarray(wt).astype(ml_dtypes.bfloat16),
                "bqk": np.ascontiguousarray(bqk).astype(np.float32),
            }
        )
    res = bass_utils.run_bass_kernel_spmd(
        nc, in_maps, core_ids=list(range(NCORES)), trace=_trace
    )
    const_row = (b_V @ W_O + b_O).astype(np.float32)  # exact: sum(softmax)=1
    out = np.zeros((B, S, D), np.float32)
    for b in range(B):
        acc = res.results[b * GROUPS]["out"].astype(np.float32)
        for g in range(1, GROUPS):
            acc = acc + res.results[b * GROUPS + g]["out"].astype(np.float32)
        out[b] = acc + const_row
    if _trace:
        kernel.last_results = res
    return out
